# revision 16
# baseline (speedup 1.0000x reference)
"""Banded local-linear layer (nn_LocalLinearLayer) on 8 trn2 NeuronCores.

out[b, o, c] = sum_p W[o, p] * xpad[b, c, p] + bias[o],  band p in [o, o+25)
xpad = concat(x[:12], x, x[-12:]) along L (first/last 12 rows duplicated).

Strategy (v5, tensor-parallel over L, single matmul per tile):
  - Each core owns 512 output rows (L/8); free dim = all B*C = 2048 cols.
    Weights are L-sharded so replicated-weight HBM traffic stays tiny.
  - Output tiled in M=104-row tiles (5 per core: 4x104 + 1x96); tile t
    contracts over xpad rows [104t, 104t+128) -> ONE K=128 matmul per
    512-col PSUM chunk (PE streams = 20x512 cols/core, the minimum for
    a 1.2GHz-throttled PE; a K-split variant costs 2x PE time).
  - Host pre-shuffles x into partition-aligned tile layout (23% duplicate
    HBM bytes, traded for halved PE stream time).
  - fp16 operands + fp16 output (fp32 PSUM accum, fp32 bias).
  - PSUM->SBUF + bias alternates ScalarE activation / VectorE tensor_scalar;
    x loads on the Sync HWDGE ring, weights on the Scalar ring, output
    stores on the GpSimd SWDGE ring so no engine queue is oversubscribed.
  - Per-tile out-DMA issued as soon as the tile's 4 chunks are copied,
    overlapping remaining input DMA + compute.
"""

import sys

for _p in ("/opt/trn_rl_repo",):
    if _p not in sys.path:
        sys.path.insert(0, _p)

import numpy as np

import concourse.bass as bass
import concourse.tile as tile
from concourse import bacc, mybir
from concourse.bass_utils import run_bass_kernel_spmd

L = 4096
WIN = 25
PAD = (WIN - 1) // 2  # 12
PADDED = L + 2 * PAD  # 4120
B = 32
C = 64
NCORES = 8
P = 128
RPC = L // NCORES  # 512 output rows per core
M = 104  # output rows per tile (K = M + WIN - 1 = 128)
NT = (RPC + M - 1) // M  # 5 tiles per core
M_LAST = RPC - (NT - 1) * M  # 96
K_LAST = M_LAST + WIN - 1  # 120
NF = B * C  # 2048 free columns
NCH = 4
CHUNK = NF // NCH  # 512 (one PSUM bank of fp32)

F32 = mybir.dt.float32
F16 = mybir.dt.float16


def _host_weights(W: np.ndarray, b: np.ndarray):
    """w1[c][k, t, m] = Wm[base+m, base+k], bias[c][m, t] = b[base+m],
    base = 512c + 104t."""
    o = np.arange(L)[:, None]
    p = np.arange(PADDED)[None, :]
    Wm = np.where((p >= o) & (p < o + WIN), W, 0.0).astype(np.float32)
    w1 = np.zeros((NCORES, P, NT, M), np.float16)
    bias = np.zeros((NCORES, M, NT), np.float32)
    for c in range(NCORES):
        for t in range(NT):
            base = RPC * c + M * t
            mt = M if t < NT - 1 else M_LAST
            kt = P if t < NT - 1 else K_LAST
            w1[c, :kt, t, :mt] = Wm[base : base + mt, base : base + kt].T
            bias[c, :mt, t] = b[base : base + mt]
    return w1, bias


def _host_x(x: np.ndarray):
    """x [B, L, C] f32 -> per-core [P, NT, NF] f16 tile layout,
    xc[c][k, t, f] = xpad[b, 512c + 104t + k, ch]  (f = 64b + ch)."""
    xp = np.concatenate([x[:, :PAD], x, x[:, -PAD:]], axis=1).astype(np.float16)
    xcs = []
    for c in range(NCORES):
        xc = np.zeros((P, NT, NF), np.float16)
        for t in range(NT):
            base = RPC * c + M * t
            kt = P if t < NT - 1 else K_LAST
            xc[:kt, t] = xp[:, base : base + kt].transpose(1, 0, 2).reshape(kt, NF)
        xcs.append(xc)
    return xcs


def _build_nc():
    nc = bacc.Bacc("TRN2", target_bir_lowering=False, debug=False, num_devices=NCORES)
    xm_d = nc.dram_tensor("xm", [P, NT, NF], F16, kind="ExternalInput").ap()
    w1_d = nc.dram_tensor("w1", [P, NT, M], F16, kind="ExternalInput").ap()
    bias_d = nc.dram_tensor("bias", [M, NT], F32, kind="ExternalInput").ap()
    out_d = nc.dram_tensor("out", [M, NT, NF], F16, kind="ExternalOutput").ap()

    with tile.TileContext(nc) as tc:
        with (
            tc.tile_pool(name="main", bufs=1) as pool,
            tc.tile_pool(name="ps", bufs=8, space=bass.MemorySpace.PSUM) as pspool,
        ):
            w1_s = pool.tile([P, NT, M], F16)
            bias_s = pool.tile([M, NT], F32)
            scr = pool.tile([P, CHUNK], F16)
            xs = [pool.tile([P, NF], F16, name=f"x{t}") for t in range(NT)]
            obs = [pool.tile([M, NF], F16, name=f"o{t}") for t in range(NT)]

            nc.sync.dma_start(xs[0][:], xm_d[:, 0, :])
            nc.sync.dma_start(w1_s[:], w1_d)
            for t in range(1, NT):
                nc.sync.dma_start(xs[t][:], xm_d[:, t, :])
            nc.scalar.dma_start(bias_s[:], bias_d)

            # PE HAM warm-up while the x stream is in flight: ~3.4us of
            # dummy matmuls, sized to drain just before x0+w1 land so the
            # real matmuls are never delayed but usually start at 2.4GHz.
            nc.gpsimd.memset(scr[:], 0.0)
            for _ in range(8):
                wps = pspool.tile([M, CHUNK], F32, name="ps")
                nc.tensor.matmul(wps[:], scr[:, :M], scr[:], start=True, stop=True)

            for t in range(NT):
                mt = M if t < NT - 1 else M_LAST
                kt = P if t < NT - 1 else K_LAST
                for j in range(NCH):
                    sl = slice(j * CHUNK, (j + 1) * CHUNK)
                    ps = pspool.tile([M, CHUNK], F32)
                    nc.tensor.matmul(
                        ps[:mt],
                        w1_s[:kt, t, :mt],
                        xs[t][:kt, sl],
                        start=True,
                        stop=True,
                    )
                    # copies balanced ACT 10 : DVE 10; stores ride the
                    # Sync ring FIFO behind the x loads
                    if j % 2 == 0:
                        nc.scalar.activation(
                            obs[t][:mt, sl],
                            ps[:mt],
                            mybir.ActivationFunctionType.Identity,
                            bias=bias_s[:mt, t : t + 1],
                        )
                    else:
                        nc.vector.tensor_scalar_add(
                            obs[t][:mt, sl], ps[:mt], bias_s[:mt, t : t + 1]
                        )
                nc.sync.dma_start(out_d[:mt, t, :], obs[t][:mt, :])

    nc.compile()
    return nc


_NC = None


def _get_nc():
    global _NC
    if _NC is None:
        _NC = _build_nc()
    return _NC


def _make_in_maps(x, W, b):
    w1, bias = _host_weights(
        np.asarray(W, dtype=np.float32), np.asarray(b, dtype=np.float32)
    )
    xcs = _host_x(np.asarray(x, dtype=np.float32))
    return [
        {"xm": xcs[c], "w1": w1[c], "bias": bias[c]}
        for c in range(NCORES)
    ]


def _gather(results):
    out = np.empty((B, L, C), np.float32)
    for c in range(NCORES):
        oh = results[c]["out"].astype(np.float32)  # [M, NT, NF]
        for t in range(NT):
            base = RPC * c + M * t
            mt = M if t < NT - 1 else M_LAST
            # [mt, B, C] -> [B, mt, C]
            out[:, base : base + mt] = (
                oh[:mt, t].reshape(mt, B, C).transpose(1, 0, 2)
            )
    return out


def kernel(x: np.ndarray, W: np.ndarray, b: np.ndarray) -> np.ndarray:
    nc = _get_nc()
    res = run_bass_kernel_spmd(nc, _make_in_maps(x, W, b), list(range(NCORES)))
    return _gather(res.results)


if __name__ == "__main__":
    rng = np.random.default_rng(0)
    x = rng.standard_normal((B, L, C), dtype=np.float32)
    W = rng.standard_normal((L, PADDED), dtype=np.float32) * 0.02
    b = rng.standard_normal((L,), dtype=np.float32) * 0.02
    print(kernel(x, W, b).shape)


# revision 17
# speedup vs baseline: 1.0529x; 1.0529x over previous
"""Banded local-linear layer (nn_LocalLinearLayer) on 8 trn2 NeuronCores.

out[b, o, c] = sum_p W[o, p] * xpad[b, c, p] + bias[o],  band p in [o, o+25)
xpad = concat(x[:12], x, x[-12:]) along L (first/last 12 rows duplicated).

Strategy (v5, tensor-parallel over L, single matmul per tile):
  - Each core owns 512 output rows (L/8); free dim = all B*C = 2048 cols.
    Weights are L-sharded so replicated-weight HBM traffic stays tiny.
  - Output tiled in M=104-row tiles (5 per core: 4x104 + 1x96); tile t
    contracts over xpad rows [104t, 104t+128) -> ONE K=128 matmul per
    512-col PSUM chunk (PE streams = 20x512 cols/core, the minimum for
    a 1.2GHz-throttled PE; a K-split variant costs 2x PE time).
  - Host pre-shuffles x into partition-aligned tile layout (23% duplicate
    HBM bytes, traded for halved PE stream time).
  - fp16 operands + fp16 output (fp32 PSUM accum, fp32 bias).
  - PSUM->SBUF + bias alternates ScalarE activation / VectorE tensor_scalar;
    x loads on the Sync HWDGE ring, weights on the Scalar ring, output
    stores on the GpSimd SWDGE ring so no engine queue is oversubscribed.
  - Per-tile out-DMA issued as soon as the tile's 4 chunks are copied,
    overlapping remaining input DMA + compute.
"""

import sys

for _p in ("/opt/trn_rl_repo",):
    if _p not in sys.path:
        sys.path.insert(0, _p)

import numpy as np

import concourse.bass as bass
import concourse.tile as tile
from concourse import bacc, mybir
from concourse.bass_utils import run_bass_kernel_spmd

L = 4096
WIN = 25
PAD = (WIN - 1) // 2  # 12
PADDED = L + 2 * PAD  # 4120
B = 32
C = 64
NCORES = 8
P = 128
RPC = L // NCORES  # 512 output rows per core
M = 104  # output rows per tile (K = M + WIN - 1 = 128)
NT = (RPC + M - 1) // M  # 5 tiles per core
M_LAST = RPC - (NT - 1) * M  # 96
K_LAST = M_LAST + WIN - 1  # 120
NF = B * C  # 2048 free columns
NCH = 4
CHUNK = NF // NCH  # 512 (one PSUM bank of fp32)

F32 = mybir.dt.float32
F16 = mybir.dt.float16


def _host_weights(W: np.ndarray, b: np.ndarray):
    """w1[c][k, t, m] = Wm[base+m, base+k], bias[c][m, t] = b[base+m],
    base = 512c + 104t."""
    o = np.arange(L)[:, None]
    p = np.arange(PADDED)[None, :]
    Wm = np.where((p >= o) & (p < o + WIN), W, 0.0).astype(np.float32)
    w1 = np.zeros((NCORES, P, NT, M), np.float16)
    bias = np.zeros((NCORES, M, NT), np.float32)
    for c in range(NCORES):
        for t in range(NT):
            base = RPC * c + M * t
            mt = M if t < NT - 1 else M_LAST
            kt = P if t < NT - 1 else K_LAST
            w1[c, :kt, t, :mt] = Wm[base : base + mt, base : base + kt].T
            bias[c, :mt, t] = b[base : base + mt]
    return w1, bias


def _host_x(x: np.ndarray):
    """x [B, L, C] f32 -> per-core [P, NT, NF] f16 tile layout,
    xc[c][k, t, f] = xpad[b, 512c + 104t + k, ch]  (f = 64b + ch)."""
    xp = np.concatenate([x[:, :PAD], x, x[:, -PAD:]], axis=1).astype(np.float16)
    xcs = []
    for c in range(NCORES):
        xc = np.zeros((P, NT, NF), np.float16)
        for t in range(NT):
            base = RPC * c + M * t
            kt = P if t < NT - 1 else K_LAST
            xc[:kt, t] = xp[:, base : base + kt].transpose(1, 0, 2).reshape(kt, NF)
        xcs.append(xc)
    return xcs


def _build_nc():
    nc = bacc.Bacc("TRN2", target_bir_lowering=False, debug=False, num_devices=NCORES)
    xm_d = nc.dram_tensor("xm", [P, NT, NF], F16, kind="ExternalInput").ap()
    w1_d = nc.dram_tensor("w1", [P, NT, M], F16, kind="ExternalInput").ap()
    bias_d = nc.dram_tensor("bias", [M, NT], F32, kind="ExternalInput").ap()
    out_d = nc.dram_tensor("out", [M, NT, NF], F16, kind="ExternalOutput").ap()

    with tile.TileContext(nc) as tc:
        with (
            tc.tile_pool(name="main", bufs=1) as pool,
            tc.tile_pool(name="ps", bufs=8, space=bass.MemorySpace.PSUM) as pspool,
        ):
            w1_s = pool.tile([P, NT, M], F16)
            bias_s = pool.tile([M, NT], F32)
            xs = [pool.tile([P, NF], F16, name=f"x{t}") for t in range(NT)]
            obs = [pool.tile([M, NF], F16, name=f"o{t}") for t in range(NT)]

            # w1 is small (131KB) and needed by the first matmul: load it
            # first, then x0 in two halves so tile 0's first chunks can
            # start ~1us earlier.
            nc.sync.dma_start(w1_s[:], w1_d)
            nc.sync.dma_start(xs[0][:, : NF // 2], xm_d[:, 0, : NF // 2])
            nc.sync.dma_start(xs[0][:, NF // 2 :], xm_d[:, 0, NF // 2 :])
            for t in range(1, NT):
                nc.sync.dma_start(xs[t][:], xm_d[:, t, :])
            nc.scalar.dma_start(bias_s[:], bias_d)

            for t in range(NT):
                mt = M if t < NT - 1 else M_LAST
                kt = P if t < NT - 1 else K_LAST
                for j in range(NCH):
                    sl = slice(j * CHUNK, (j + 1) * CHUNK)
                    ps = pspool.tile([M, CHUNK], F32)
                    nc.tensor.matmul(
                        ps[:mt],
                        w1_s[:kt, t, :mt],
                        xs[t][:kt, sl],
                        start=True,
                        stop=True,
                    )
                    # copies balanced ACT 10 : DVE 10; stores ride the
                    # Sync ring FIFO behind the x loads
                    if j % 2 == 0:
                        nc.scalar.activation(
                            obs[t][:mt, sl],
                            ps[:mt],
                            mybir.ActivationFunctionType.Identity,
                            bias=bias_s[:mt, t : t + 1],
                        )
                    else:
                        nc.vector.tensor_scalar_add(
                            obs[t][:mt, sl], ps[:mt], bias_s[:mt, t : t + 1]
                        )
                nc.sync.dma_start(out_d[:mt, t, :], obs[t][:mt, :])

    nc.compile()
    return nc


_NC = None


def _get_nc():
    global _NC
    if _NC is None:
        _NC = _build_nc()
    return _NC


def _make_in_maps(x, W, b):
    w1, bias = _host_weights(
        np.asarray(W, dtype=np.float32), np.asarray(b, dtype=np.float32)
    )
    xcs = _host_x(np.asarray(x, dtype=np.float32))
    return [
        {"xm": xcs[c], "w1": w1[c], "bias": bias[c]}
        for c in range(NCORES)
    ]


def _gather(results):
    out = np.empty((B, L, C), np.float32)
    for c in range(NCORES):
        oh = results[c]["out"].astype(np.float32)  # [M, NT, NF]
        for t in range(NT):
            base = RPC * c + M * t
            mt = M if t < NT - 1 else M_LAST
            # [mt, B, C] -> [B, mt, C]
            out[:, base : base + mt] = (
                oh[:mt, t].reshape(mt, B, C).transpose(1, 0, 2)
            )
    return out


def kernel(x: np.ndarray, W: np.ndarray, b: np.ndarray) -> np.ndarray:
    nc = _get_nc()
    res = run_bass_kernel_spmd(nc, _make_in_maps(x, W, b), list(range(NCORES)))
    return _gather(res.results)


if __name__ == "__main__":
    rng = np.random.default_rng(0)
    x = rng.standard_normal((B, L, C), dtype=np.float32)
    W = rng.standard_normal((L, PADDED), dtype=np.float32) * 0.02
    b = rng.standard_normal((L,), dtype=np.float32) * 0.02
    print(kernel(x, W, b).shape)


# revision 20
# speedup vs baseline: 1.1202x; 1.0639x over previous
"""Banded local-linear layer (nn_LocalLinearLayer) on 8 trn2 NeuronCores.

out[b, o, c] = sum_p W[o, p] * xpad[b, c, p] + bias[o],  band p in [o, o+25)
xpad = concat(x[:12], x, x[-12:]) along L (first/last 12 rows duplicated).

Strategy (v11, tensor-parallel over L, single matmul per tile):
  - Each core owns 512 output rows (L/8); free dim = all B*C = 2048 cols.
    Weights are L-sharded so replicated-weight HBM traffic stays tiny
    (~135KB/core vs 1MB+ for batch-parallel).
  - Output tiled in M=104-row tiles (5 per core: 4x104 + 1x96); tile t
    contracts over xpad rows [104t, 104t+128) -> ONE K=128 matmul per
    512-col PSUM chunk (20x512-col PE streams/core, the minimum for the
    often-1.2GHz-throttled PE; a K=128+24 split costs ~2x PE time and
    measured slower even warm).
  - Host pre-shuffles x into partition-aligned tile layout (23% duplicate
    HBM bytes, traded for halved PE stream time).
  - fp16 operands + fp16 output (fp32 PSUM accum, fp32 bias): halves
    output HBM bytes vs f32.
  - PSUM->SBUF + bias alternates ScalarE activation / VectorE
    tensor_scalar per 512-col chunk (10/10 split, bufs=8 PSUM banks).
  - x0 loads first, then w1, then x1..x4 and the per-tile output stores
    all on the Sync HWDGE ring (FIFO keeps the 16 SDMA engines saturated:
    stores drain right after the input stream); bias rides the idle
    Scalar ring so its tiny descriptors don't stall the main ring.
"""

import sys

for _p in ("/opt/trn_rl_repo",):
    if _p not in sys.path:
        sys.path.insert(0, _p)

import numpy as np

import concourse.bass as bass
import concourse.tile as tile
from concourse import bacc, mybir
from concourse.bass_utils import run_bass_kernel_spmd

L = 4096
WIN = 25
PAD = (WIN - 1) // 2  # 12
PADDED = L + 2 * PAD  # 4120
B = 32
C = 64
NCORES = 8
P = 128
RPC = L // NCORES  # 512 output rows per core
M = 104  # output rows per tile (K = M + WIN - 1 = 128)
NT = (RPC + M - 1) // M  # 5 tiles per core
M_LAST = RPC - (NT - 1) * M  # 96
K_LAST = M_LAST + WIN - 1  # 120
NF = B * C  # 2048 free columns
NCH = 4
CHUNK = NF // NCH  # 512 (one PSUM bank of fp32)

F32 = mybir.dt.float32
F16 = mybir.dt.float16


def _host_weights(W: np.ndarray, b: np.ndarray):
    """w1[c][k, t, m] = Wm[base+m, base+k], bias[c][m, t] = b[base+m],
    base = 512c + 104t."""
    o = np.arange(L)[:, None]
    p = np.arange(PADDED)[None, :]
    Wm = np.where((p >= o) & (p < o + WIN), W, 0.0).astype(np.float32)
    w1 = np.zeros((NCORES, P, NT, M), np.float16)
    bias = np.zeros((NCORES, M, NT), np.float32)
    for c in range(NCORES):
        for t in range(NT):
            base = RPC * c + M * t
            mt = M if t < NT - 1 else M_LAST
            kt = P if t < NT - 1 else K_LAST
            w1[c, :kt, t, :mt] = Wm[base : base + mt, base : base + kt].T
            bias[c, :mt, t] = b[base : base + mt]
    return w1, bias


def _host_x(x: np.ndarray):
    """x [B, L, C] f32 -> per-core [P, NT, NF] f16 tile layout,
    xc[c][k, t, f] = xpad[b, 512c + 104t + k, ch]  (f = 64b + ch)."""
    xp = np.concatenate([x[:, :PAD], x, x[:, -PAD:]], axis=1).astype(np.float16)
    xcs = []
    for c in range(NCORES):
        xc = np.zeros((P, NT, NF), np.float16)
        for t in range(NT):
            base = RPC * c + M * t
            kt = P if t < NT - 1 else K_LAST
            xc[:kt, t] = xp[:, base : base + kt].transpose(1, 0, 2).reshape(kt, NF)
        xcs.append(xc)
    return xcs


def _build_nc():
    nc = bacc.Bacc("TRN2", target_bir_lowering=False, debug=False, num_devices=NCORES)
    xm_d = nc.dram_tensor("xm", [P, NT, NF], F16, kind="ExternalInput").ap()
    w1_d = nc.dram_tensor("w1", [P, NT, M], F16, kind="ExternalInput").ap()
    bias_d = nc.dram_tensor("bias", [M, NT], F32, kind="ExternalInput").ap()
    out_d = nc.dram_tensor("out", [M, NT, NF], F16, kind="ExternalOutput").ap()

    with tile.TileContext(nc) as tc:
        with (
            tc.tile_pool(name="main", bufs=1) as pool,
            tc.tile_pool(name="ps", bufs=8, space=bass.MemorySpace.PSUM) as pspool,
        ):
            w1_s = pool.tile([P, NT, M], F16)
            bias_s = pool.tile([M, NT], F32)
            xs = [pool.tile([P, NF], F16, name=f"x{t}") for t in range(NT)]
            obs = [pool.tile([M, NF], F16, name=f"o{t}") for t in range(NT)]

            nc.sync.dma_start(xs[0][:], xm_d[:, 0, :])
            nc.sync.dma_start(w1_s[:], w1_d)
            for t in range(1, NT):
                nc.sync.dma_start(xs[t][:], xm_d[:, t, :])
            nc.scalar.dma_start(bias_s[:], bias_d)

            for t in range(NT):
                mt = M if t < NT - 1 else M_LAST
                kt = P if t < NT - 1 else K_LAST
                for j in range(NCH):
                    sl = slice(j * CHUNK, (j + 1) * CHUNK)
                    ps = pspool.tile([M, CHUNK], F32)
                    nc.tensor.matmul(
                        ps[:mt],
                        w1_s[:kt, t, :mt],
                        xs[t][:kt, sl],
                        start=True,
                        stop=True,
                    )
                    # copies balanced ACT 10 : DVE 10; stores ride the
                    # Sync ring FIFO behind the x loads
                    if j % 2 == 0:
                        nc.scalar.activation(
                            obs[t][:mt, sl],
                            ps[:mt],
                            mybir.ActivationFunctionType.Identity,
                            bias=bias_s[:mt, t : t + 1],
                        )
                    else:
                        nc.vector.tensor_scalar_add(
                            obs[t][:mt, sl], ps[:mt], bias_s[:mt, t : t + 1]
                        )
                # per-half-tile stores: each 256KB half leaves as soon as
                # its two chunks are copied, shortening the out-stream tail
                nc.sync.dma_start(
                    out_d[:mt, t, : NF // 2], obs[t][:mt, : NF // 2]
                )
                nc.sync.dma_start(
                    out_d[:mt, t, NF // 2 :], obs[t][:mt, NF // 2 :]
                )

    nc.compile()
    return nc


_NC = None


def _get_nc():
    global _NC
    if _NC is None:
        _NC = _build_nc()
    return _NC


def _make_in_maps(x, W, b):
    w1, bias = _host_weights(
        np.asarray(W, dtype=np.float32), np.asarray(b, dtype=np.float32)
    )
    xcs = _host_x(np.asarray(x, dtype=np.float32))
    return [
        {"xm": xcs[c], "w1": w1[c], "bias": bias[c]}
        for c in range(NCORES)
    ]


def _gather(results):
    out = np.empty((B, L, C), np.float32)
    for c in range(NCORES):
        oh = results[c]["out"].astype(np.float32)  # [M, NT, NF]
        for t in range(NT):
            base = RPC * c + M * t
            mt = M if t < NT - 1 else M_LAST
            # [mt, B, C] -> [B, mt, C]
            out[:, base : base + mt] = (
                oh[:mt, t].reshape(mt, B, C).transpose(1, 0, 2)
            )
    return out


def kernel(x: np.ndarray, W: np.ndarray, b: np.ndarray) -> np.ndarray:
    nc = _get_nc()
    res = run_bass_kernel_spmd(nc, _make_in_maps(x, W, b), list(range(NCORES)))
    return _gather(res.results)


if __name__ == "__main__":
    rng = np.random.default_rng(0)
    x = rng.standard_normal((B, L, C), dtype=np.float32)
    W = rng.standard_normal((L, PADDED), dtype=np.float32) * 0.02
    b = rng.standard_normal((L,), dtype=np.float32) * 0.02
    print(kernel(x, W, b).shape)
